# revision 10
# baseline (speedup 1.0000x reference)
"""Trainium2 Bass kernel for nn_Memory_58025008169465 (scatter_memory).

Data-parallel over N = B*H*W = 16384 query rows across 8 NeuronCores
(2 batches per core).  Keys are replicated.  Cross-core reductions:
  AR1 (add): colsum of exp(score) [1024]
  AR2 (max): colmax of exp(score) [128,8]
  AR3 (add): segment-sum query_update [1024,512] + similarity-loss scalar
"""

import sys

_BASS_ROOT = "/opt/trn_rl_repo"
if _BASS_ROOT not in sys.path:
    sys.path.insert(0, _BASS_ROOT)

import numpy as np

import concourse.bass as bass
import concourse.bacc as bacc
import concourse.mybir as mybir
import concourse.tile as tile
from concourse import bass_utils, masks

dt = mybir.dt
AF = mybir.ActivationFunctionType
ALU = mybir.AluOpType
AX = mybir.AxisListType

B, C, H, W, M = 16, 512, 32, 32, 1024
HW = H * W           # 1024
N = B * HW           # 16384
NCORES = 8
BL = B // NCORES     # 2 batches per core
NL = BL * HW         # 2048 rows per core
NCH = NL // 128      # 16 n-chunks
CCH = C // 128       # 4 c-chunks
MCH = M // 128       # 8 m-chunks

EPS_NORM = 1e-12
EPS_COS = 1e-8

# matmul dtypes (float32r = fast fp32 on PE; bf16 where precision allows)
SCORE_DT = dt.float32r
ONES_DT = dt.float32r
G_DT = dt.float32r

F32 = dt.float32
BF16 = dt.bfloat16


def _mm(nc, out, lhsT, rhs, start, stop, mm_dt=None):
    if mm_dt is not None:
        lhsT = lhsT.bitcast(mm_dt)
        rhs = rhs.bitcast(mm_dt)
    nc.tensor.matmul(out, lhsT, rhs, start=start, stop=stop)


def build():
    nc = bacc.Bacc("TRN2", target_bir_lowering=False, debug=False,
                   num_devices=NCORES)

    q_in = nc.dram_tensor("q", [BL, C, H, W], F32, kind="ExternalInput").ap()
    keys_in = nc.dram_tensor("keys", [M, C], F32, kind="ExternalInput").ap()
    ent_in = nc.dram_tensor("ent", [NL, 1], F32, kind="ExternalInput").ap()
    entf_in = nc.dram_tensor("entf", [N, 1], F32, kind="ExternalInput").ap()

    uq_out = nc.dram_tensor("uq", [BL, 2 * C, H, W], F32, kind="ExternalOutput").ap()
    um_out = nc.dram_tensor("um", [M, C], F32, kind="ExternalOutput").ap()
    sq_out = nc.dram_tensor("sq", [NL, M], F32, kind="ExternalOutput").ap()
    sm_out = nc.dram_tensor("sm", [NL, M], F32, kind="ExternalOutput").ap()
    dl_out = nc.dram_tensor("dl", [1, 1], F32, kind="ExternalOutput").ap()
    sl_out = nc.dram_tensor("sl", [1, 1], F32, kind="ExternalOutput").ap()

    with tile.TileContext(nc) as tc:
        _body(nc, tc, q_in, keys_in, ent_in, entf_in,
              uq_out, um_out, sq_out, sm_out, dl_out, sl_out)
    nc.compile()
    return nc


def _body(nc, tc, q_in, keys_in, ent_in, entf_in,
          uq_out, um_out, sq_out, sm_out, dl_out, sl_out):
    from contextlib import ExitStack
    ctx = ExitStack()
    with ctx:
        consts = ctx.enter_context(tc.tile_pool(name="consts", bufs=1))
        stats = ctx.enter_context(tc.tile_pool(name="stats", bufs=1))
        dram = ctx.enter_context(tc.tile_pool(name="dram", bufs=1, space="DRAM"))
        ps_small = ctx.enter_context(
            tc.tile_pool(name="ps_small", bufs=2, space="PSUM"))

        def ps_s_tile(shape, name):
            return ps_small.tile(shape, F32, name=name, tag="s")

        ident = consts.tile([128, 128], F32, name="ident")
        masks.make_identity(nc, ident[:])
        ones_col = consts.tile([128, 1], F32, name="ones_col")
        nc.vector.memset(ones_col[:], 1.0)
        one1 = consts.tile([1, 1], F32, name="one1")
        nc.vector.memset(one1[:], 1.0)
        ones_row = consts.tile([1, 128], F32, name="ones_row")
        nc.vector.memset(ones_row[:], 1.0)

        # ---------- keys ----------
        keys_nat = consts.tile([128, MCH * C], F32, name="keys_nat")
        for mj in range(MCH):
            nc.sync.dma_start(keys_nat[:, mj * C:(mj + 1) * C],
                              keys_in[mj * 128:(mj + 1) * 128, :])
        keys_bf = consts.tile([128, MCH * C], BF16, name="keys_bf")
        for mj in range(MCH):
            nc.vector.tensor_copy(keys_bf[:, mj * C:(mj + 1) * C],
                                  keys_nat[:, mj * C:(mj + 1) * C])
        # inverse keynorm (per m, partition layout [128, MCH])
        ksq_scr = stats.tile([128, C], F32, name="ksq_scr")
        ikn_p = stats.tile([128, MCH], F32, name="ikn_p")
        for mj in range(MCH):
            nc.scalar.activation(ksq_scr[:], keys_nat[:, mj * C:(mj + 1) * C],
                                 AF.Square, accum_out=ikn_p[:, mj:mj + 1])
        nc.scalar.activation(ikn_p[:], ikn_p[:], AF.Sqrt)
        nc.vector.tensor_scalar_max(ikn_p[:], ikn_p[:], EPS_COS)
        nc.vector.reciprocal(ikn_p[:], ikn_p[:])

        # ---------- entropy ----------
        ent_l = stats.tile([128, NCH], F32, name="ent_l")
        nc.sync.dma_start(ent_l[:], ent_in.rearrange("(i p) o -> p (i o)", p=128))
        entf_sb = stats.tile([128, N // 128], F32, name="entf_sb")
        nc.sync.dma_start(entf_sb[:], entf_in.rearrange("(i p) o -> p (i o)", p=128))
        em_part = stats.tile([128, 1], F32, name="em_part")
        nc.vector.tensor_reduce(em_part[:], entf_sb[:], axis=AX.X, op=ALU.max)
        em_row_ps = ps_s_tile([1, 128], "em_row_ps")
        _mm(nc, em_row_ps[:], em_part[:], ident[:, :], True, True)
        em_row = stats.tile([1, 128], F32, name="em_row")
        nc.scalar.copy(em_row[:], em_row_ps[:])
        em1 = stats.tile([1, 1], F32, name="em1")
        nc.vector.tensor_reduce(em1[:], em_row[:], axis=AX.X, op=ALU.max)
        rem1 = stats.tile([1, 1], F32, name="rem1")
        nc.vector.reciprocal(rem1[:], em1[:])
        rem_ps = ps_s_tile([128, 1], "rem_ps")
        _mm(nc, rem_ps[:], ones_row[:], rem1[:], True, True)
        rem_b = stats.tile([128, 1], F32, name="rem_b")
        nc.scalar.copy(rem_b[:], rem_ps[:])
        ent_rsum = stats.tile([128, 1], F32, name="ent_rsum")
        nc.vector.tensor_reduce(ent_rsum[:], ent_l[:], axis=AX.X, op=ALU.add)
        entsum_ps = ps_s_tile([1, 1], "entsum_ps")
        _mm(nc, entsum_ps[:], ones_col[:], ent_rsum[:], True, True)
        entsum1 = stats.tile([1, 1], F32, name="entsum1")
        nc.scalar.copy(entsum1[:], entsum_ps[:])

        # ---------- q: load, sumsq, normalize (both layouts) ----------
        qrB_bf = consts.tile([128, NCH * C], BF16, name="qrB_bf")
        inv_p = stats.tile([128, NCH], F32, name="inv_p")
        rowsum2 = stats.tile([128, 2 * NCH], F32, name="rowsum2")
        rowsumE = stats.tile([128, NCH], F32, name="rowsumE")
        rowmaxE = stats.tile([128, NCH], F32, name="rowmaxE")
        E_big = None

        with tc.tile_pool(name="pA", bufs=1) as pA:
            keysT = pA.tile([128, CCH * M], dt.float32r, name="keysT")
            qrT_r = pA.tile([128, CCH * NL], dt.float32r, name="qrT_r")

            with tc.tile_pool(name="pA_tmp", bufs=1) as pAt:
                qT_raw = pAt.tile([128, CCH * NL], F32, name="qT_raw")
                for cj in range(CCH):
                    for b in range(BL):
                        nc.sync.dma_start(
                            qT_raw[:, cj * NL + b * HW: cj * NL + (b + 1) * HW],
                            q_in[b, cj * 128:(cj + 1) * 128, :, :]
                            .rearrange("c h w -> c (h w)"))
                inv_f = pAt.tile([1, NL], F32, name="inv_f")
                with tc.tile_pool(name="qsq_str", bufs=2) as pQs, \
                     tc.tile_pool(name="ps_ssq", bufs=4, space="PSUM") as ps_ssq:
                    ssq_tiles = [ps_ssq.tile([1, 512], F32, name=f"ssq{nk}",
                                             tag="ssq")
                                 for nk in range(NL // 512)]
                    for cj in range(CCH):
                        qsq = pQs.tile([128, NL], F32, name="qsq")
                        nc.scalar.activation(qsq[:],
                                             qT_raw[:, cj * NL:(cj + 1) * NL],
                                             AF.Square)
                        for nk in range(NL // 512):
                            _mm(nc, ssq_tiles[nk][:], ones_col[:],
                                qsq[:, nk * 512:(nk + 1) * 512],
                                cj == 0, cj == CCH - 1)
                    for nk in range(NL // 512):
                        nc.scalar.copy(inv_f[:, nk * 512:(nk + 1) * 512],
                                       ssq_tiles[nk][:])
                nc.scalar.activation(inv_f[:], inv_f[:], AF.Sqrt)
                nc.vector.tensor_scalar_max(inv_f[:], inv_f[:], EPS_NORM)
                nc.vector.reciprocal(inv_f[:], inv_f[:])

                with tc.tile_pool(name="ps_t", bufs=4, space="PSUM") as ps_t:
                    def ps_t_tile(shape, name):
                        return ps_t.tile(shape, F32, name=name, tag="t")

                    # invnorm partition layout [128, NCH]
                    invp_ps = ps_t_tile([128, NCH], "invp_ps")
                    for i in range(NCH):
                        _mm(nc, invp_ps[:, i:i + 1],
                            inv_f[:, i * 128:(i + 1) * 128], one1[:], True, True)
                    nc.scalar.copy(inv_p[:], invp_ps[:])
                    # invnorm free-broadcast [128, NL]
                    inb = pAt.tile([128, NL], F32, name="inb")
                    for nk in range(NL // 512):
                        inb_ps = ps_t_tile([128, 512], "inb_ps")
                        _mm(nc, inb_ps[:], ones_row[:],
                            inv_f[:, nk * 512:(nk + 1) * 512], True, True)
                        nc.scalar.copy(inb[:, nk * 512:(nk + 1) * 512], inb_ps[:])
                    # qrT = qT_raw * inb (f32r); write uq first half
                    for cj in range(CCH):
                        nc.vector.scalar_tensor_tensor(
                            qrT_r[:, cj * NL:(cj + 1) * NL],
                            qT_raw[:, cj * NL:(cj + 1) * NL], 1.0, inb[:],
                            op0=ALU.mult, op1=ALU.mult)
                        for b in range(BL):
                            nc.sync.dma_start(
                                uq_out[b, cj * 128:(cj + 1) * 128, :, :]
                                .rearrange("c h w -> c (h w)"),
                                qrT_r[:, cj * NL + b * HW:
                                      cj * NL + (b + 1) * HW].bitcast(F32))
                    # layout B: transpose raw q, scale by inv_p, cast bf16
                    for i in range(NCH):
                        qb_ps = ps_t_tile([128, C], "qb_ps")
                        for cj in range(CCH):
                            nc.tensor.transpose(
                                qb_ps[:, cj * 128:(cj + 1) * 128],
                                qT_raw[:, cj * NL + i * 128: cj * NL + (i + 1) * 128],
                                ident[:])
                        qb_raw = pAt.tile([128, C], F32, name="qb_raw", bufs=2)
                        nc.scalar.copy(qb_raw[:], qb_ps[:])
                        nc.vector.tensor_scalar(
                            out=qrB_bf[:, i * C:(i + 1) * C], in0=qb_raw[:],
                            scalar1=inv_p[:, i:i + 1], scalar2=None, op0=ALU.mult)
                    # keysT [c-part, m] via PE transpose
                    for cj in range(CCH):
                        for mj in range(MCH):
                            kt_ps = ps_t_tile([128, 128], "kt_ps")
                            nc.tensor.transpose(
                                kt_ps[:],
                                keys_nat[:, mj * C + cj * 128:
                                         mj * C + (cj + 1) * 128],
                                ident[:])
                            nc.scalar.copy(
                                keysT[:, cj * M + mj * 128: cj * M + (mj + 1) * 128],
                                kt_ps[:])

            # ---------- score matmul (layout B) + exp ----------
            E_pool = tc.alloc_tile_pool(name="E_pool", bufs=1, side="right")
            E_big = E_pool.tile([128, NCH * M], F32, name="E_big")

            with tc.tile_pool(name="ps_sc", bufs=2, space="PSUM") as ps_sc:
                for i in range(NCH):
                    for mh in range(2):
                        sc_ps = ps_sc.tile([128, 512], F32, name="sc_ps", tag="sc")
                        for cj in range(CCH):
                            _mm(nc, sc_ps[:],
                                qrT_r[:, cj * NL + i * 128: cj * NL + (i + 1) * 128],
                                keysT[:, cj * M + mh * 512: cj * M + (mh + 1) * 512],
                                cj == 0, cj == CCH - 1)
                        nc.scalar.activation(
                            E_big[:, i * M + mh * 512: i * M + (mh + 1) * 512],
                            sc_ps[:], AF.Exp,
                            accum_out=rowsum2[:, 2 * i + mh: 2 * i + mh + 1])

        # ---------- row stats ----------
        nc.vector.tensor_tensor(rowsumE[:], rowsum2[:, 0:2 * NCH:2],
                                rowsum2[:, 1:2 * NCH:2], ALU.add)
        rrow = stats.tile([128, NCH], F32, name="rrow")
        nc.vector.reciprocal(rrow[:], rowsumE[:])
        for i in range(NCH):
            nc.vector.tensor_reduce(rowmaxE[:, i:i + 1],
                                    E_big[:, i * M:(i + 1) * M],
                                    axis=AX.X, op=ALU.max)

        # ---------- sm output + sm bf16 to dram ----------
        smd = dram.tile([NL, M], BF16, name="smd")
        with tc.tile_pool(name="sm_str", bufs=3) as pS:
            for i in range(NCH):
                sm_t = pS.tile([128, M], F32, name="sm_t")
                nc.vector.tensor_scalar(
                    out=sm_t[:], in0=E_big[:, i * M:(i + 1) * M],
                    scalar1=rrow[:, i:i + 1], scalar2=None, op0=ALU.mult)
                nc.sync.dma_start(sm_out[i * 128:(i + 1) * 128, :], sm_t[:])
                smb_t = pS.tile([128, M], BF16, name="smb_t")
                nc.vector.tensor_scalar(
                    out=smb_t[:], in0=E_big[:, i * M:(i + 1) * M],
                    scalar1=rrow[:, i:i + 1], scalar2=None, op0=ALU.mult)
                nc.sync.dma_start(smd[i * 128:(i + 1) * 128, :], smb_t[:])

        # ---------- colsum (ones-matmul) -> AR1 ----------
        cs_l = stats.tile([1, M], F32, name="cs_l")
        with tc.tile_pool(name="ps_cs", bufs=1, space="PSUM") as ps_cs:
            cs_ps = ps_cs.tile([1, M], F32, name="cs_ps")
            for mh in range(2):
                for i in range(NCH):
                    _mm(nc, cs_ps[:, mh * 512:(mh + 1) * 512], ones_col[:],
                        E_big[:, i * M + mh * 512: i * M + (mh + 1) * 512],
                        i == 0, i == NCH - 1)
            nc.scalar.copy(cs_l[:], cs_ps[:])
        ar1_in = dram.tile([1, M], F32, name="ar1_in")
        ar1_out = dram.tile([1, M], F32, name="ar1_out", addr_space="Shared")
        nc.sync.dma_start(ar1_in[:], cs_l[:])
        nc.gpsimd.collective_compute(
            "AllReduce", ALU.add, replica_groups=[list(range(NCORES))],
            ins=[ar1_in.opt()], outs=[ar1_out.opt()])

        # ---------- colmax (gpsimd chain + PE transpose) -> AR2 ----------
        cmx = stats.tile([128, M], F32, name="cmx")
        cmx2 = stats.tile([128, M], F32, name="cmx2")
        nc.vector.tensor_tensor(cmx[:], E_big[:, 0:M], E_big[:, M:2 * M], ALU.max)
        nc.vector.tensor_tensor(cmx2[:], E_big[:, 2 * M:3 * M],
                                E_big[:, 3 * M:4 * M], ALU.max)
        for i in range(4, NCH, 2):
            nc.vector.tensor_tensor(cmx[:], cmx[:],
                                    E_big[:, i * M:(i + 1) * M], ALU.max)
            nc.vector.tensor_tensor(cmx2[:], cmx2[:],
                                    E_big[:, (i + 1) * M:(i + 2) * M], ALU.max)
        nc.vector.tensor_tensor(cmx[:], cmx[:], cmx2[:], ALU.max)
        cm_l = stats.tile([128, MCH], F32, name="cm_l")
        cm_sb = stats.tile([128, 128], F32, name="cm_sb")
        with tc.tile_pool(name="ps_cm", bufs=2, space="PSUM") as ps_cm:
            for mj in range(MCH):
                cm_ps = ps_cm.tile([128, 128], F32, name="cm_ps", tag="cm")
                nc.tensor.transpose(cm_ps[:], cmx[:, mj * 128:(mj + 1) * 128],
                                    ident[:])
                nc.scalar.copy(cm_sb[:], cm_ps[:])
                nc.vector.tensor_reduce(cm_l[:, mj:mj + 1], cm_sb[:],
                                        axis=AX.X, op=ALU.max)
        ar2_in = dram.tile([128, MCH], F32, name="ar2_in")
        ar2_out = dram.tile([128, MCH], F32, name="ar2_out", addr_space="Shared")
        nc.sync.dma_start(ar2_in[:], cm_l[:])
        nc.gpsimd.collective_compute(
            "AllReduce", ALU.max, replica_groups=[list(range(NCORES))],
            ins=[ar2_in.opt()], outs=[ar2_out.opt()])

        # ---------- smT (bf16 DMA transpose) + read matmul ----------
        with tc.tile_pool(name="p_read", bufs=1) as pRd, \
             tc.tile_pool(name="rT_pool", bufs=2) as pR, \
             tc.tile_pool(name="ps_r", bufs=2, space="PSUM") as ps_r:
            smT = pRd.tile([128, MCH * NL], BF16, name="smT")
            for mj in range(MCH):
                nc.sync.dma_start_transpose(
                    smT[:, mj * NL:(mj + 1) * NL],
                    smd[:, mj * 128:(mj + 1) * 128])
            for cj in range(CCH):
                rT = pR.tile([128, NL], F32, name="rT")
                for nk in range(NL // 512):
                    r_ps = ps_r.tile([128, 512], F32, name="r_ps", tag="r")
                    for mj in range(MCH):
                        _mm(nc, r_ps[:],
                            keys_bf[:, mj * C + cj * 128: mj * C + (cj + 1) * 128],
                            smT[:, mj * NL + nk * 512: mj * NL + (nk + 1) * 512],
                            mj == 0, mj == MCH - 1)
                    nc.scalar.copy(rT[:, nk * 512:(nk + 1) * 512], r_ps[:])
                for b in range(BL):
                    nc.sync.dma_start(
                        uq_out[b, C + cj * 128: C + (cj + 1) * 128, :, :]
                        .rearrange("c h w -> c (h w)"),
                        rT[:, b * HW:(b + 1) * HW])

        # ---------- after AR2/AR1: broadcast tiles ----------
        cm_g = stats.tile([128, MCH], F32, name="cm_g")
        nc.sync.dma_start(cm_g[:], ar2_out[:])
        icm_p = stats.tile([128, MCH], F32, name="icm_p")
        nc.vector.reciprocal(icm_p[:], cm_g[:])
        bcast = tc.alloc_tile_pool(name="bcast", bufs=1, side="right")
        icm_b = bcast.tile([128, M], F32, name="icm_b")
        ics_b = bcast.tile([128, M], F32, name="ics_b")
        with tc.tile_pool(name="ps_b", bufs=2, space="PSUM") as ps_b:
            icm_f_ps = ps_b.tile([1, M], F32, name="icm_f_ps", bufs=1, tag="bw")
            for mj in range(MCH):
                _mm(nc, icm_f_ps[:, mj * 128:(mj + 1) * 128],
                    icm_p[:, mj:mj + 1], ident[:], True, True)
            icm_f = stats.tile([1, M], F32, name="icm_f")
            nc.scalar.copy(icm_f[:], icm_f_ps[:])
            cs_g = stats.tile([1, M], F32, name="cs_g")
            nc.sync.dma_start(cs_g[:], ar1_out[:])
            ics_f = stats.tile([1, M], F32, name="ics_f")
            nc.vector.reciprocal(ics_f[:], cs_g[:])
            for mh in range(2):
                ib_ps = ps_b.tile([128, 512], F32, name="ib_ps", tag="bb")
                _mm(nc, ib_ps[:], ones_row[:], icm_f[:, mh * 512:(mh + 1) * 512],
                    True, True)
                nc.scalar.copy(icm_b[:, mh * 512:(mh + 1) * 512], ib_ps[:])
                ib2_ps = ps_b.tile([128, 512], F32, name="ib2_ps", tag="bb")
                _mm(nc, ib2_ps[:], ones_row[:], ics_f[:, mh * 512:(mh + 1) * 512],
                    True, True)
                nc.scalar.copy(ics_b[:, mh * 512:(mh + 1) * 512], ib2_ps[:])

        # per-row factors
        b_f = stats.tile([128, NCH], F32, name="b_f")
        nc.vector.tensor_tensor(b_f[:], rowmaxE[:], ent_l[:], ALU.mult)
        nc.vector.tensor_scalar(out=b_f[:], in0=b_f[:], scalar1=rem_b[:, 0:1],
                                scalar2=None, op0=ALU.mult)
        a_f = stats.tile([128, NCH], F32, name="a_f")
        nc.scalar.activation(a_f[:], rowmaxE[:], AF.Ln)
        nc.vector.tensor_tensor(a_f[:], a_f[:], ent_l[:], ALU.mult)

        # ---------- S', rhs, sq output ----------
        seg = tc.alloc_tile_pool(name="seg", bufs=1, side="right")
        Sp = seg.tile([128, NCH * M], BF16, name="Sp")
        rhsM = seg.tile([128, NCH * C], BF16, name="rhsM")
        rhsA = seg.tile([128, NCH * 8], BF16, name="rhsA")
        nc.vector.memset(rhsA[:], 0.0)
        with tc.tile_pool(name="sq_str", bufs=3) as pQ:
            for i in range(NCH):
                nc.vector.scalar_tensor_tensor(
                    Sp[:, i * M:(i + 1) * M], E_big[:, i * M:(i + 1) * M],
                    rowmaxE[:, i:i + 1], icm_b[:],
                    op0=ALU.is_equal, op1=ALU.mult)
                nc.vector.tensor_scalar(
                    out=rhsM[:, i * C:(i + 1) * C],
                    in0=qrB_bf[:, i * C:(i + 1) * C],
                    scalar1=b_f[:, i:i + 1], scalar2=None, op0=ALU.mult)
                nc.vector.tensor_copy(rhsA[:, i * 8:i * 8 + 1], a_f[:, i:i + 1])
                sq_t = pQ.tile([128, M], F32, name="sq_t")
                nc.vector.tensor_tensor(sq_t[:], E_big[:, i * M:(i + 1) * M],
                                        ics_b[:], ALU.mult)
                nc.sync.dma_start(sq_out[i * 128:(i + 1) * 128, :], sq_t[:])

        # ---------- segment matmul -> AR3 ----------
        ar3_in = dram.tile([M + 1, C + 4], F32, name="ar3_in")
        ar3_out = dram.tile([M + 1, C + 4], F32, name="ar3_out", addr_space="Shared")
        asum_l = stats.tile([128, MCH], F32, name="asum_l")
        with tc.tile_pool(name="qu_str", bufs=3) as pU, \
             tc.tile_pool(name="ps_m", bufs=2, space="PSUM") as ps_m, \
             tc.tile_pool(name="ps_x", bufs=2, space="PSUM") as ps_x:
            for mj in range(MCH):
                qu_ps = ps_m.tile([128, C], F32, name="qu_ps", tag="m")
                ax_ps = ps_x.tile([128, 8], F32, name="ax_ps", tag="x")
                for i in range(NCH):
                    lhs = Sp[:, i * M + mj * 128: i * M + (mj + 1) * 128]
                    _mm(nc, qu_ps[:], lhs, rhsM[:, i * C:(i + 1) * C],
                        i == 0, i == NCH - 1)
                    _mm(nc, ax_ps[:], lhs, rhsA[:, i * 8:(i + 1) * 8],
                        i == 0, i == NCH - 1)
                qu_sb = pU.tile([128, C + 4], F32, name="qu_sb")
                nc.scalar.copy(qu_sb[:, 0:C], qu_ps[:])
                nc.scalar.copy(qu_sb[:, C:C + 4], ax_ps[:, 0:4])
                nc.vector.tensor_copy(asum_l[:, mj:mj + 1], ax_ps[:, 0:1])
                nc.sync.dma_start(ar3_in[mj * 128:(mj + 1) * 128, :], qu_sb[:])
            # similarity partial: entsum - sum_m asum*cm*ikn
            s2t = stats.tile([128, MCH], F32, name="s2t")
            nc.vector.tensor_tensor(s2t[:], asum_l[:], cm_g[:], ALU.mult)
            nc.vector.tensor_tensor(s2t[:], s2t[:], ikn_p[:], ALU.mult)
            s2r = stats.tile([128, 1], F32, name="s2r")
            nc.vector.tensor_reduce(s2r[:], s2t[:], axis=AX.X, op=ALU.add)
            s2_ps = ps_x.tile([1, 1], F32, name="s2_ps", bufs=1, tag="s2")
            _mm(nc, s2_ps[:], ones_col[:], s2r[:], True, True)
            sim_row = pU.tile([1, C + 4], F32, name="sim_row")
            nc.vector.memset(sim_row[:], 0.0)
            nc.vector.tensor_tensor(sim_row[:, 0:1], entsum1[:], s2_ps[:],
                                    ALU.subtract)
            nc.sync.dma_start(ar3_in[M:M + 1, :], sim_row[:])
        nc.gpsimd.collective_compute(
            "AllReduce", ALU.add, replica_groups=[list(range(NCORES))],
            ins=[ar3_in.opt()], outs=[ar3_out.opt()])
        seg.release()
        bcast.release()
        E_pool.release()

        # ---------- tail: updated memory + gram loss ----------
        with tc.tile_pool(name="tail", bufs=1) as pT, \
             tc.tile_pool(name="tail_str", bufs=3) as pTs, \
             tc.tile_pool(name="ps_g", bufs=2, space="PSUM") as ps_g:
            U_big = pT.tile([128, MCH * C], F32, name="U_big")
            zs = stats.tile([128, MCH], F32, name="zs")
            for mj in range(MCH):
                qu_g = pTs.tile([128, C + 4], F32, name="qu_g")
                nc.sync.dma_start(qu_g[:], ar3_out[mj * 128:(mj + 1) * 128, :])
                nc.vector.tensor_tensor(U_big[:, mj * C:(mj + 1) * C],
                                        qu_g[:, 0:C],
                                        keys_nat[:, mj * C:(mj + 1) * C], ALU.add)
            usq_scr = stats.tile([128, C], F32, name="usq_scr")
            iun = stats.tile([128, MCH], F32, name="iun")
            for mj in range(MCH):
                nc.scalar.activation(usq_scr[:], U_big[:, mj * C:(mj + 1) * C],
                                     AF.Square, accum_out=iun[:, mj:mj + 1])
            nc.scalar.activation(iun[:], iun[:], AF.Sqrt)
            nc.vector.tensor_scalar_max(iun[:], iun[:], EPS_NORM)
            nc.vector.reciprocal(iun[:], iun[:])
            for mj in range(MCH):
                nc.vector.tensor_scalar(
                    out=U_big[:, mj * C:(mj + 1) * C],
                    in0=U_big[:, mj * C:(mj + 1) * C],
                    scalar1=iun[:, mj:mj + 1], scalar2=None, op0=ALU.mult)
                nc.sync.dma_start(um_out[mj * 128:(mj + 1) * 128, :],
                                  U_big[:, mj * C:(mj + 1) * C])
                nc.scalar.activation(usq_scr[:], U_big[:, mj * C:(mj + 1) * C],
                                     AF.Square, accum_out=zs[:, mj:mj + 1])
            # G = U^T U [C, C]; SG = sum(G^2); dloss = (SG - sum z^2)/(M^2-M)
            U_bf = pT.tile([128, MCH * C], BF16, name="U_bf")
            for mj in range(MCH):
                nc.vector.tensor_copy(U_bf[:, mj * C:(mj + 1) * C],
                                      U_big[:, mj * C:(mj + 1) * C])
            sg_part = stats.tile([128, CCH], F32, name="sg_part")
            g_scr = stats.tile([128, C], F32, name="g_scr")
            for cj in range(CCH):
                g_ps = ps_g.tile([128, C], F32, name="g_ps", tag="g")
                for mj in range(MCH):
                    _mm(nc, g_ps[:],
                        U_bf[:, mj * C + cj * 128: mj * C + (cj + 1) * 128],
                        U_bf[:, mj * C:(mj + 1) * C],
                        mj == 0, mj == MCH - 1)
                nc.scalar.activation(g_scr[:], g_ps[:], AF.Square,
                                     accum_out=sg_part[:, cj:cj + 1])
            nc.vector.tensor_tensor(zs[:], zs[:], zs[:], ALU.mult)
            zred = stats.tile([128, 1], F32, name="zred")
            nc.vector.tensor_reduce(zred[:], zs[:], axis=AX.X, op=ALU.add)
            sgred = stats.tile([128, 1], F32, name="sgred")
            nc.vector.tensor_reduce(sgred[:], sg_part[:], axis=AX.X, op=ALU.add)
            nc.vector.tensor_tensor(sgred[:], sgred[:], zred[:], ALU.subtract)
            dl_ps = ps_g.tile([1, 1], F32, name="dl_ps", tag="g")
            _mm(nc, dl_ps[:], ones_col[:], sgred[:], True, True)
            dl_sb = stats.tile([1, 1], F32, name="dl_sb")
            nc.scalar.mul(dl_sb[:], dl_ps[:], 1.0 / (M * M - M))
            nc.sync.dma_start(dl_out[:, :], dl_sb[:])
            sl_sb = stats.tile([1, 1], F32, name="sl_sb")
            nc.sync.dma_start(sl_sb[:], ar3_out[M:M + 1, 0:1])
            nc.sync.dma_start(sl_out[:, :], sl_sb[:])


_NC_CACHE = None


def kernel(query, keys, entropy):
    global _NC_CACHE
    if _NC_CACHE is None:
        _NC_CACHE = build()
    nc = _NC_CACHE
    query = np.ascontiguousarray(query, dtype=np.float32)
    keys = np.ascontiguousarray(keys, dtype=np.float32)
    entropy = np.ascontiguousarray(entropy, dtype=np.float32)
    in_maps = []
    for c in range(NCORES):
        in_maps.append({
            "q": query[c * BL:(c + 1) * BL],
            "keys": keys,
            "ent": entropy[c * NL:(c + 1) * NL],
            "entf": entropy,
        })
    res = bass_utils.run_bass_kernel_spmd(nc, in_maps,
                                          core_ids=list(range(NCORES)))
    r = res.results
    uq = np.concatenate([r[c]["uq"] for c in range(NCORES)], axis=0)
    sq = np.concatenate([r[c]["sq"] for c in range(NCORES)], axis=0)
    sm = np.concatenate([r[c]["sm"] for c in range(NCORES)], axis=0)
    um = r[0]["um"]
    dl = np.float32(r[0]["dl"][0, 0])
    sl = np.float32(r[0]["sl"][0, 0])
    return uq, um, sq, sm, dl, sl


# revision 12
# speedup vs baseline: 181.9607x; 181.9607x over previous
"""Trainium2 Bass kernel for nn_Memory_58025008169465 (scatter_memory).

Data-parallel over N = B*H*W = 16384 query rows across 8 NeuronCores
(2 batches per core).  Keys are replicated.  Cross-core reductions:
  AR1 (add): colsum of exp(score) [1024]
  AR2 (max): colmax of exp(score) [128,8]
  AR3 (add): segment-sum query_update [1024,512] + similarity-loss scalar
"""

import sys

_BASS_ROOT = "/opt/trn_rl_repo"
if _BASS_ROOT not in sys.path:
    sys.path.insert(0, _BASS_ROOT)

import numpy as np

import concourse.bass as bass
import concourse.bacc as bacc
import concourse.mybir as mybir
import concourse.tile as tile
from concourse import bass_utils, masks

dt = mybir.dt
AF = mybir.ActivationFunctionType
ALU = mybir.AluOpType
AX = mybir.AxisListType

B, C, H, W, M = 16, 512, 32, 32, 1024
HW = H * W           # 1024
N = B * HW           # 16384
NCORES = 8
BL = B // NCORES     # 2 batches per core
NL = BL * HW         # 2048 rows per core
NCH = NL // 128      # 16 n-chunks
CCH = C // 128       # 4 c-chunks
MCH = M // 128       # 8 m-chunks

EPS_NORM = 1e-12
EPS_COS = 1e-8

# matmul dtypes (float32r = fast fp32 on PE; bf16 where precision allows)
SCORE_DT = dt.float32r
ONES_DT = dt.float32r
G_DT = dt.float32r

F32 = dt.float32
BF16 = dt.bfloat16


def _mm(nc, out, lhsT, rhs, start, stop, mm_dt=None):
    if mm_dt is not None:
        lhsT = lhsT.bitcast(mm_dt)
        rhs = rhs.bitcast(mm_dt)
    nc.tensor.matmul(out, lhsT, rhs, start=start, stop=stop)


def build():
    nc = bacc.Bacc("TRN2", target_bir_lowering=False, debug=False,
                   num_devices=NCORES)

    q_in = nc.dram_tensor("q", [BL, C, H, W], F32, kind="ExternalInput").ap()
    keys_in = nc.dram_tensor("keys", [M, C], F32, kind="ExternalInput").ap()
    ent_in = nc.dram_tensor("ent", [NL, 1], F32, kind="ExternalInput").ap()
    entf_in = nc.dram_tensor("entf", [N, 1], F32, kind="ExternalInput").ap()

    uq_out = nc.dram_tensor("uq", [BL, 2 * C, H, W], F32, kind="ExternalOutput").ap()
    um_out = nc.dram_tensor("um", [M, C], F32, kind="ExternalOutput").ap()
    sq_out = nc.dram_tensor("sq", [NL, M], F32, kind="ExternalOutput").ap()
    sm_out = nc.dram_tensor("sm", [NL, M], F32, kind="ExternalOutput").ap()
    dl_out = nc.dram_tensor("dl", [1, 1], F32, kind="ExternalOutput").ap()
    sl_out = nc.dram_tensor("sl", [1, 1], F32, kind="ExternalOutput").ap()

    with tile.TileContext(nc) as tc:
        _body(nc, tc, q_in, keys_in, ent_in, entf_in,
              uq_out, um_out, sq_out, sm_out, dl_out, sl_out)
    nc.compile()
    return nc


def _body(nc, tc, q_in, keys_in, ent_in, entf_in,
          uq_out, um_out, sq_out, sm_out, dl_out, sl_out):
    from contextlib import ExitStack
    ctx = ExitStack()
    with ctx:
        consts = ctx.enter_context(tc.tile_pool(name="consts", bufs=1))
        stats = ctx.enter_context(tc.tile_pool(name="stats", bufs=1))
        dram = ctx.enter_context(tc.tile_pool(name="dram", bufs=1, space="DRAM"))
        ps_small = ctx.enter_context(
            tc.tile_pool(name="ps_small", bufs=2, space="PSUM"))

        def ps_s_tile(shape, name):
            return ps_small.tile(shape, F32, name=name, tag="s")

        ident = consts.tile([128, 128], F32, name="ident")
        masks.make_identity(nc, ident[:])
        ones_col = consts.tile([128, 1], F32, name="ones_col")
        nc.vector.memset(ones_col[:], 1.0)
        one1 = consts.tile([1, 1], F32, name="one1")
        nc.vector.memset(one1[:], 1.0)
        ones_row = consts.tile([1, 128], F32, name="ones_row")
        nc.vector.memset(ones_row[:], 1.0)

        # ---------- keys ----------
        keys_nat = consts.tile([128, MCH * C], F32, name="keys_nat")
        for mj in range(MCH):
            nc.sync.dma_start(keys_nat[:, mj * C:(mj + 1) * C],
                              keys_in[mj * 128:(mj + 1) * 128, :])
        keys_bf = consts.tile([128, MCH * C], BF16, name="keys_bf")
        for mj in range(MCH):
            nc.vector.tensor_copy(keys_bf[:, mj * C:(mj + 1) * C],
                                  keys_nat[:, mj * C:(mj + 1) * C])
        # inverse keynorm (per m, partition layout [128, MCH])
        ksq_scr = stats.tile([128, C], F32, name="ksq_scr")
        ikn_p = stats.tile([128, MCH], F32, name="ikn_p")
        for mj in range(MCH):
            nc.scalar.activation(ksq_scr[:], keys_nat[:, mj * C:(mj + 1) * C],
                                 AF.Square, accum_out=ikn_p[:, mj:mj + 1])
        nc.scalar.activation(ikn_p[:], ikn_p[:], AF.Sqrt)
        nc.vector.tensor_scalar_max(ikn_p[:], ikn_p[:], EPS_COS)
        nc.vector.reciprocal(ikn_p[:], ikn_p[:])

        # ---------- entropy ----------
        ent_l = stats.tile([128, NCH], F32, name="ent_l")
        nc.sync.dma_start(ent_l[:], ent_in.rearrange("(i p) o -> p (i o)", p=128))
        entf_sb = stats.tile([128, N // 128], F32, name="entf_sb")
        nc.sync.dma_start(entf_sb[:], entf_in.rearrange("(i p) o -> p (i o)", p=128))
        em_part = stats.tile([128, 1], F32, name="em_part")
        nc.vector.tensor_reduce(em_part[:], entf_sb[:], axis=AX.X, op=ALU.max)
        em_row_ps = ps_s_tile([1, 128], "em_row_ps")
        _mm(nc, em_row_ps[:], em_part[:], ident[:, :], True, True)
        em_row = stats.tile([1, 128], F32, name="em_row")
        nc.scalar.copy(em_row[:], em_row_ps[:])
        em1 = stats.tile([1, 1], F32, name="em1")
        nc.vector.tensor_reduce(em1[:], em_row[:], axis=AX.X, op=ALU.max)
        rem1 = stats.tile([1, 1], F32, name="rem1")
        nc.vector.reciprocal(rem1[:], em1[:])
        rem_ps = ps_s_tile([128, 1], "rem_ps")
        _mm(nc, rem_ps[:], ones_row[:], rem1[:], True, True)
        rem_b = stats.tile([128, 1], F32, name="rem_b")
        nc.scalar.copy(rem_b[:], rem_ps[:])
        ent_rsum = stats.tile([128, 1], F32, name="ent_rsum")
        nc.vector.tensor_reduce(ent_rsum[:], ent_l[:], axis=AX.X, op=ALU.add)
        entsum_ps = ps_s_tile([1, 1], "entsum_ps")
        _mm(nc, entsum_ps[:], ones_col[:], ent_rsum[:], True, True)
        entsum1 = stats.tile([1, 1], F32, name="entsum1")
        nc.scalar.copy(entsum1[:], entsum_ps[:])

        # ---------- q: load, sumsq, normalize (both layouts) ----------
        qrB_bf = consts.tile([128, NCH * C], BF16, name="qrB_bf")
        inv_p = stats.tile([128, NCH], F32, name="inv_p")
        rowsum2 = stats.tile([128, 2 * NCH], F32, name="rowsum2")
        rowsumE = stats.tile([128, NCH], F32, name="rowsumE")
        rowmaxE = stats.tile([128, NCH], F32, name="rowmaxE")
        E_big = None

        with tc.tile_pool(name="pA", bufs=1) as pA:
            keysT = pA.tile([128, CCH * M], dt.float32r, name="keysT")
            qrT_r = pA.tile([128, CCH * NL], dt.float32r, name="qrT_r")

            with tc.tile_pool(name="pA_tmp", bufs=1) as pAt:
                qT_raw = pAt.tile([128, CCH * NL], F32, name="qT_raw")
                for cj in range(CCH):
                    for b in range(BL):
                        nc.sync.dma_start(
                            qT_raw[:, cj * NL + b * HW: cj * NL + (b + 1) * HW],
                            q_in[b, cj * 128:(cj + 1) * 128, :, :]
                            .rearrange("c h w -> c (h w)"))
                inv_f = pAt.tile([1, NL], F32, name="inv_f")
                with tc.tile_pool(name="qsq_str", bufs=2) as pQs, \
                     tc.tile_pool(name="ps_ssq", bufs=4, space="PSUM") as ps_ssq:
                    ssq_tiles = [ps_ssq.tile([1, 512], F32, name=f"ssq{nk}",
                                             tag="ssq")
                                 for nk in range(NL // 512)]
                    for cj in range(CCH):
                        qsq = pQs.tile([128, NL], F32, name="qsq")
                        nc.scalar.activation(qsq[:],
                                             qT_raw[:, cj * NL:(cj + 1) * NL],
                                             AF.Square)
                        for nk in range(NL // 512):
                            _mm(nc, ssq_tiles[nk][:], ones_col[:],
                                qsq[:, nk * 512:(nk + 1) * 512],
                                cj == 0, cj == CCH - 1)
                    for nk in range(NL // 512):
                        nc.scalar.copy(inv_f[:, nk * 512:(nk + 1) * 512],
                                       ssq_tiles[nk][:])
                nc.scalar.activation(inv_f[:], inv_f[:], AF.Sqrt)
                nc.vector.tensor_scalar_max(inv_f[:], inv_f[:], EPS_NORM)
                nc.vector.reciprocal(inv_f[:], inv_f[:])

                with tc.tile_pool(name="ps_t", bufs=4, space="PSUM") as ps_t:
                    def ps_t_tile(shape, name):
                        return ps_t.tile(shape, F32, name=name, tag="t")

                    # invnorm partition layout [128, NCH]
                    invp_ps = ps_t_tile([128, NCH], "invp_ps")
                    for i in range(NCH):
                        _mm(nc, invp_ps[:, i:i + 1],
                            inv_f[:, i * 128:(i + 1) * 128], one1[:], True, True)
                    nc.scalar.copy(inv_p[:], invp_ps[:])
                    # invnorm free-broadcast [128, NL]
                    inb = pAt.tile([128, NL], F32, name="inb")
                    for nk in range(NL // 512):
                        inb_ps = ps_t_tile([128, 512], "inb_ps")
                        _mm(nc, inb_ps[:], ones_row[:],
                            inv_f[:, nk * 512:(nk + 1) * 512], True, True)
                        nc.scalar.copy(inb[:, nk * 512:(nk + 1) * 512], inb_ps[:])
                    # qrT = qT_raw * inb (f32r); write uq first half
                    for cj in range(CCH):
                        nc.vector.scalar_tensor_tensor(
                            qrT_r[:, cj * NL:(cj + 1) * NL],
                            qT_raw[:, cj * NL:(cj + 1) * NL], 1.0, inb[:],
                            op0=ALU.mult, op1=ALU.mult)
                        for b in range(BL):
                            nc.sync.dma_start(
                                uq_out[b, cj * 128:(cj + 1) * 128, :, :]
                                .rearrange("c h w -> c (h w)"),
                                qrT_r[:, cj * NL + b * HW:
                                      cj * NL + (b + 1) * HW].bitcast(F32))
                    # layout B: transpose raw q, scale by inv_p, cast bf16
                    for i in range(NCH):
                        qb_ps = ps_t_tile([128, C], "qb_ps")
                        for cj in range(CCH):
                            nc.tensor.transpose(
                                qb_ps[:, cj * 128:(cj + 1) * 128],
                                qT_raw[:, cj * NL + i * 128: cj * NL + (i + 1) * 128],
                                ident[:])
                        qb_raw = pAt.tile([128, C], F32, name="qb_raw", bufs=2)
                        nc.scalar.copy(qb_raw[:], qb_ps[:])
                        nc.vector.tensor_scalar(
                            out=qrB_bf[:, i * C:(i + 1) * C], in0=qb_raw[:],
                            scalar1=inv_p[:, i:i + 1], scalar2=None, op0=ALU.mult)
                    # keysT [c-part, m] via PE transpose
                    for cj in range(CCH):
                        for mj in range(MCH):
                            kt_ps = ps_t_tile([128, 128], "kt_ps")
                            nc.tensor.transpose(
                                kt_ps[:],
                                keys_nat[:, mj * C + cj * 128:
                                         mj * C + (cj + 1) * 128],
                                ident[:])
                            nc.scalar.copy(
                                keysT[:, cj * M + mj * 128: cj * M + (mj + 1) * 128],
                                kt_ps[:])

            # ---------- score matmul (layout B) + exp ----------
            E_pool = tc.alloc_tile_pool(name="E_pool", bufs=1, side="right")
            E_big = E_pool.tile([128, NCH * M], F32, name="E_big")

            with tc.tile_pool(name="ps_sc", bufs=2, space="PSUM") as ps_sc:
                for i in range(NCH):
                    for mh in range(2):
                        sc_ps = ps_sc.tile([128, 512], F32, name="sc_ps", tag="sc")
                        for cj in range(CCH):
                            _mm(nc, sc_ps[:],
                                qrT_r[:, cj * NL + i * 128: cj * NL + (i + 1) * 128],
                                keysT[:, cj * M + mh * 512: cj * M + (mh + 1) * 512],
                                cj == 0, cj == CCH - 1)
                        nc.scalar.activation(
                            E_big[:, i * M + mh * 512: i * M + (mh + 1) * 512],
                            sc_ps[:], AF.Exp,
                            accum_out=rowsum2[:, 2 * i + mh: 2 * i + mh + 1])

        # ---------- row stats ----------
        nc.vector.tensor_tensor(rowsumE[:], rowsum2[:, 0:2 * NCH:2],
                                rowsum2[:, 1:2 * NCH:2], ALU.add)
        rrow = stats.tile([128, NCH], F32, name="rrow")
        nc.vector.reciprocal(rrow[:], rowsumE[:])
        for i in range(NCH):
            nc.vector.tensor_reduce(rowmaxE[:, i:i + 1],
                                    E_big[:, i * M:(i + 1) * M],
                                    axis=AX.X, op=ALU.max)

        # ---------- sm output + sm bf16 to dram ----------
        smd = dram.tile([NL, M], BF16, name="smd")
        with tc.tile_pool(name="sm_str", bufs=3) as pS:
            for i in range(NCH):
                sm_t = pS.tile([128, M], F32, name="sm_t")
                nc.vector.tensor_scalar(
                    out=sm_t[:], in0=E_big[:, i * M:(i + 1) * M],
                    scalar1=rrow[:, i:i + 1], scalar2=None, op0=ALU.mult)
                nc.sync.dma_start(sm_out[i * 128:(i + 1) * 128, :], sm_t[:])
                smb_t = pS.tile([128, M], BF16, name="smb_t")
                nc.vector.tensor_scalar(
                    out=smb_t[:], in0=E_big[:, i * M:(i + 1) * M],
                    scalar1=rrow[:, i:i + 1], scalar2=None, op0=ALU.mult)
                nc.sync.dma_start(smd[i * 128:(i + 1) * 128, :], smb_t[:])

        # ---------- colsum (ones-matmul) -> AR1 ----------
        cs_l = stats.tile([1, M], F32, name="cs_l")
        with tc.tile_pool(name="ps_cs", bufs=1, space="PSUM") as ps_cs:
            cs_ps = ps_cs.tile([1, M], F32, name="cs_ps")
            for mh in range(2):
                for i in range(NCH):
                    _mm(nc, cs_ps[:, mh * 512:(mh + 1) * 512], ones_col[:],
                        E_big[:, i * M + mh * 512: i * M + (mh + 1) * 512],
                        i == 0, i == NCH - 1)
            nc.scalar.copy(cs_l[:], cs_ps[:])
        ar1_in = dram.tile([1, M], F32, name="ar1_in")
        ar1_out = dram.tile([1, M], F32, name="ar1_out", addr_space="Shared")
        nc.sync.dma_start(ar1_in[:], cs_l[:])
        nc.gpsimd.collective_compute(
            "AllReduce", ALU.add, replica_groups=[list(range(NCORES))],
            ins=[ar1_in.opt()], outs=[ar1_out.opt()])

        # ---------- colmax (gpsimd chain + PE transpose) -> AR2 ----------
        cmx = stats.tile([128, M], F32, name="cmx")
        cmx2 = stats.tile([128, M], F32, name="cmx2")
        nc.vector.tensor_tensor(cmx[:], E_big[:, 0:M], E_big[:, M:2 * M], ALU.max)
        nc.vector.tensor_tensor(cmx2[:], E_big[:, 2 * M:3 * M],
                                E_big[:, 3 * M:4 * M], ALU.max)
        for i in range(4, NCH, 2):
            nc.vector.tensor_tensor(cmx[:], cmx[:],
                                    E_big[:, i * M:(i + 1) * M], ALU.max)
            nc.vector.tensor_tensor(cmx2[:], cmx2[:],
                                    E_big[:, (i + 1) * M:(i + 2) * M], ALU.max)
        nc.vector.tensor_tensor(cmx[:], cmx[:], cmx2[:], ALU.max)
        cm_l = stats.tile([128, MCH], F32, name="cm_l")
        cm_sb = stats.tile([128, 128], F32, name="cm_sb")
        with tc.tile_pool(name="ps_cm", bufs=2, space="PSUM") as ps_cm:
            for mj in range(MCH):
                cm_ps = ps_cm.tile([128, 128], F32, name="cm_ps", tag="cm")
                nc.tensor.transpose(cm_ps[:], cmx[:, mj * 128:(mj + 1) * 128],
                                    ident[:])
                nc.scalar.copy(cm_sb[:], cm_ps[:])
                nc.vector.tensor_reduce(cm_l[:, mj:mj + 1], cm_sb[:],
                                        axis=AX.X, op=ALU.max)
        ar2_in = dram.tile([128, MCH], F32, name="ar2_in")
        ar2_out = dram.tile([128, MCH], F32, name="ar2_out", addr_space="Shared")
        nc.sync.dma_start(ar2_in[:], cm_l[:])
        nc.gpsimd.collective_compute(
            "AllReduce", ALU.max, replica_groups=[list(range(NCORES))],
            ins=[ar2_in.opt()], outs=[ar2_out.opt()])

        # ---------- smT (bf16 DMA transpose) + read matmul ----------
        with tc.tile_pool(name="p_read", bufs=1) as pRd, \
             tc.tile_pool(name="rT_pool", bufs=2) as pR, \
             tc.tile_pool(name="ps_r", bufs=2, space="PSUM") as ps_r:
            smT = pRd.tile([128, MCH * NL], BF16, name="smT")
            for mj in range(MCH):
                nc.sync.dma_start_transpose(
                    smT[:, mj * NL:(mj + 1) * NL],
                    smd[:, mj * 128:(mj + 1) * 128])
            for cj in range(CCH):
                rT = pR.tile([128, NL], F32, name="rT")
                for nk in range(NL // 512):
                    r_ps = ps_r.tile([128, 512], F32, name="r_ps", tag="r")
                    for mj in range(MCH):
                        _mm(nc, r_ps[:],
                            keys_bf[:, mj * C + cj * 128: mj * C + (cj + 1) * 128],
                            smT[:, mj * NL + nk * 512: mj * NL + (nk + 1) * 512],
                            mj == 0, mj == MCH - 1)
                    nc.scalar.copy(rT[:, nk * 512:(nk + 1) * 512], r_ps[:])
                for b in range(BL):
                    nc.sync.dma_start(
                        uq_out[b, C + cj * 128: C + (cj + 1) * 128, :, :]
                        .rearrange("c h w -> c (h w)"),
                        rT[:, b * HW:(b + 1) * HW])

        # ---------- after AR2/AR1: broadcast tiles ----------
        cm_g = stats.tile([128, MCH], F32, name="cm_g")
        nc.sync.dma_start(cm_g[:], ar2_out[:])
        icm_p = stats.tile([128, MCH], F32, name="icm_p")
        nc.vector.reciprocal(icm_p[:], cm_g[:])
        bcast = tc.alloc_tile_pool(name="bcast", bufs=1, side="right")
        icm_b = bcast.tile([128, M], F32, name="icm_b")
        ics_b = bcast.tile([128, M], F32, name="ics_b")
        with tc.tile_pool(name="ps_b", bufs=2, space="PSUM") as ps_b:
            icm_f_ps = ps_b.tile([1, M], F32, name="icm_f_ps", bufs=1, tag="bw")
            for mj in range(MCH):
                _mm(nc, icm_f_ps[:, mj * 128:(mj + 1) * 128],
                    icm_p[:, mj:mj + 1], ident[:], True, True)
            icm_f = stats.tile([1, M], F32, name="icm_f")
            nc.scalar.copy(icm_f[:], icm_f_ps[:])
            cs_g = stats.tile([1, M], F32, name="cs_g")
            nc.sync.dma_start(cs_g[:], ar1_out[:])
            ics_f = stats.tile([1, M], F32, name="ics_f")
            nc.vector.reciprocal(ics_f[:], cs_g[:])
            for mh in range(2):
                ib_ps = ps_b.tile([128, 512], F32, name="ib_ps", tag="bb")
                _mm(nc, ib_ps[:], ones_row[:], icm_f[:, mh * 512:(mh + 1) * 512],
                    True, True)
                nc.scalar.copy(icm_b[:, mh * 512:(mh + 1) * 512], ib_ps[:])
                ib2_ps = ps_b.tile([128, 512], F32, name="ib2_ps", tag="bb")
                _mm(nc, ib2_ps[:], ones_row[:], ics_f[:, mh * 512:(mh + 1) * 512],
                    True, True)
                nc.scalar.copy(ics_b[:, mh * 512:(mh + 1) * 512], ib2_ps[:])

        # per-row factors
        b_f = stats.tile([128, NCH], F32, name="b_f")
        nc.vector.tensor_tensor(b_f[:], rowmaxE[:], ent_l[:], ALU.mult)
        nc.vector.tensor_scalar(out=b_f[:], in0=b_f[:], scalar1=rem_b[:, 0:1],
                                scalar2=None, op0=ALU.mult)
        a_f = stats.tile([128, NCH], F32, name="a_f")
        nc.scalar.activation(a_f[:], rowmaxE[:], AF.Ln)
        nc.vector.tensor_tensor(a_f[:], a_f[:], ent_l[:], ALU.mult)

        # ---------- S', rhs, sq output ----------
        seg = tc.alloc_tile_pool(name="seg", bufs=1, side="right")
        Sp = seg.tile([128, NCH * M], BF16, name="Sp")
        rhsM = seg.tile([128, NCH * C], BF16, name="rhsM")
        rhsA = seg.tile([128, NCH * 8], BF16, name="rhsA")
        nc.vector.memset(rhsA[:], 0.0)
        with tc.tile_pool(name="sq_str", bufs=3) as pQ:
            for i in range(NCH):
                nc.vector.scalar_tensor_tensor(
                    Sp[:, i * M:(i + 1) * M], E_big[:, i * M:(i + 1) * M],
                    rowmaxE[:, i:i + 1], icm_b[:],
                    op0=ALU.is_equal, op1=ALU.mult)
                nc.vector.tensor_scalar(
                    out=rhsM[:, i * C:(i + 1) * C],
                    in0=qrB_bf[:, i * C:(i + 1) * C],
                    scalar1=b_f[:, i:i + 1], scalar2=None, op0=ALU.mult)
                nc.vector.tensor_copy(rhsA[:, i * 8:i * 8 + 1], a_f[:, i:i + 1])
                sq_t = pQ.tile([128, M], F32, name="sq_t")
                nc.vector.tensor_tensor(sq_t[:], E_big[:, i * M:(i + 1) * M],
                                        ics_b[:], ALU.mult)
                nc.sync.dma_start(sq_out[i * 128:(i + 1) * 128, :], sq_t[:])

        # ---------- segment matmul -> AR3 ----------
        ar3_in = dram.tile([M + 1, C + 4], F32, name="ar3_in")
        ar3_out = dram.tile([M + 1, C + 4], F32, name="ar3_out", addr_space="Shared")
        asum_l = stats.tile([128, MCH], F32, name="asum_l")
        with tc.tile_pool(name="qu_str", bufs=3) as pU, \
             tc.tile_pool(name="ps_m", bufs=2, space="PSUM") as ps_m, \
             tc.tile_pool(name="ps_x", bufs=2, space="PSUM") as ps_x:
            for mj in range(MCH):
                qu_ps = ps_m.tile([128, C], F32, name="qu_ps", tag="m")
                ax_ps = ps_x.tile([128, 8], F32, name="ax_ps", tag="x")
                for i in range(NCH):
                    lhs = Sp[:, i * M + mj * 128: i * M + (mj + 1) * 128]
                    _mm(nc, qu_ps[:], lhs, rhsM[:, i * C:(i + 1) * C],
                        i == 0, i == NCH - 1)
                    _mm(nc, ax_ps[:], lhs, rhsA[:, i * 8:(i + 1) * 8],
                        i == 0, i == NCH - 1)
                qu_sb = pU.tile([128, C + 4], F32, name="qu_sb")
                nc.scalar.copy(qu_sb[:, 0:C], qu_ps[:])
                nc.scalar.copy(qu_sb[:, C:C + 4], ax_ps[:, 0:4])
                nc.vector.tensor_copy(asum_l[:, mj:mj + 1], ax_ps[:, 0:1])
                nc.sync.dma_start(ar3_in[mj * 128:(mj + 1) * 128, :], qu_sb[:])
            # similarity partial: entsum - sum_m asum*cm*ikn
            s2t = stats.tile([128, MCH], F32, name="s2t")
            nc.vector.tensor_tensor(s2t[:], asum_l[:], cm_g[:], ALU.mult)
            nc.vector.tensor_tensor(s2t[:], s2t[:], ikn_p[:], ALU.mult)
            s2r = stats.tile([128, 1], F32, name="s2r")
            nc.vector.tensor_reduce(s2r[:], s2t[:], axis=AX.X, op=ALU.add)
            s2_ps = ps_x.tile([1, 1], F32, name="s2_ps", bufs=1, tag="s2")
            _mm(nc, s2_ps[:], ones_col[:], s2r[:], True, True)
            sim_row = pU.tile([1, C + 4], F32, name="sim_row")
            nc.vector.memset(sim_row[:], 0.0)
            nc.vector.tensor_tensor(sim_row[:, 0:1], entsum1[:], s2_ps[:],
                                    ALU.subtract)
            nc.sync.dma_start(ar3_in[M:M + 1, :], sim_row[:])
        nc.gpsimd.collective_compute(
            "AllReduce", ALU.add, replica_groups=[list(range(NCORES))],
            ins=[ar3_in.opt()], outs=[ar3_out.opt()])
        seg.release()
        bcast.release()
        E_pool.release()

        # ---------- tail: updated memory + gram loss ----------
        with tc.tile_pool(name="tail", bufs=1) as pT, \
             tc.tile_pool(name="tail_str", bufs=3) as pTs, \
             tc.tile_pool(name="ps_g", bufs=2, space="PSUM") as ps_g:
            U_big = pT.tile([128, MCH * C], F32, name="U_big")
            zs = stats.tile([128, MCH], F32, name="zs")
            for mj in range(MCH):
                qu_g = pTs.tile([128, C + 4], F32, name="qu_g")
                nc.sync.dma_start(qu_g[:], ar3_out[mj * 128:(mj + 1) * 128, :])
                nc.vector.tensor_tensor(U_big[:, mj * C:(mj + 1) * C],
                                        qu_g[:, 0:C],
                                        keys_nat[:, mj * C:(mj + 1) * C], ALU.add)
            usq_scr = stats.tile([128, C], F32, name="usq_scr")
            iun = stats.tile([128, MCH], F32, name="iun")
            for mj in range(MCH):
                nc.scalar.activation(usq_scr[:], U_big[:, mj * C:(mj + 1) * C],
                                     AF.Square, accum_out=iun[:, mj:mj + 1])
            nc.scalar.activation(iun[:], iun[:], AF.Sqrt)
            nc.vector.tensor_scalar_max(iun[:], iun[:], EPS_NORM)
            nc.vector.reciprocal(iun[:], iun[:])
            for mj in range(MCH):
                nc.vector.tensor_scalar(
                    out=U_big[:, mj * C:(mj + 1) * C],
                    in0=U_big[:, mj * C:(mj + 1) * C],
                    scalar1=iun[:, mj:mj + 1], scalar2=None, op0=ALU.mult)
                nc.sync.dma_start(um_out[mj * 128:(mj + 1) * 128, :],
                                  U_big[:, mj * C:(mj + 1) * C])
                nc.scalar.activation(usq_scr[:], U_big[:, mj * C:(mj + 1) * C],
                                     AF.Square, accum_out=zs[:, mj:mj + 1])
            # G = U^T U [C, C]; SG = sum(G^2); dloss = (SG - sum z^2)/(M^2-M)
            U_bf = pT.tile([128, MCH * C], BF16, name="U_bf")
            for mj in range(MCH):
                nc.vector.tensor_copy(U_bf[:, mj * C:(mj + 1) * C],
                                      U_big[:, mj * C:(mj + 1) * C])
            sg_part = stats.tile([128, CCH], F32, name="sg_part")
            g_scr = stats.tile([128, C], F32, name="g_scr")
            for cj in range(CCH):
                g_ps = ps_g.tile([128, C], F32, name="g_ps", tag="g")
                for mj in range(MCH):
                    _mm(nc, g_ps[:],
                        U_bf[:, mj * C + cj * 128: mj * C + (cj + 1) * 128],
                        U_bf[:, mj * C:(mj + 1) * C],
                        mj == 0, mj == MCH - 1)
                nc.scalar.activation(g_scr[:], g_ps[:], AF.Square,
                                     accum_out=sg_part[:, cj:cj + 1])
            nc.vector.tensor_tensor(zs[:], zs[:], zs[:], ALU.mult)
            zred = stats.tile([128, 1], F32, name="zred")
            nc.vector.tensor_reduce(zred[:], zs[:], axis=AX.X, op=ALU.add)
            sgred = stats.tile([128, 1], F32, name="sgred")
            nc.vector.tensor_reduce(sgred[:], sg_part[:], axis=AX.X, op=ALU.add)
            nc.vector.tensor_tensor(sgred[:], sgred[:], zred[:], ALU.subtract)
            dl_ps = ps_g.tile([1, 1], F32, name="dl_ps", tag="g")
            _mm(nc, dl_ps[:], ones_col[:], sgred[:], True, True)
            dl_sb = stats.tile([1, 1], F32, name="dl_sb")
            nc.scalar.mul(dl_sb[:], dl_ps[:], 1.0 / (M * M - M))
            nc.sync.dma_start(dl_out[:, :], dl_sb[:])
            sl_sb = stats.tile([1, 1], F32, name="sl_sb")
            nc.sync.dma_start(sl_sb[:], ar3_out[M:M + 1, 0:1])
            nc.sync.dma_start(sl_out[:, :], sl_sb[:])


_RUNNER = None


def _make_runner():
    """Build the compiled 8-core executable once: jitted shard_map over the
    bass custom-call, plus an on-device zero-maker for donated output bufs."""
    import jax
    import jax.numpy as jnp
    from jax.sharding import Mesh, PartitionSpec
    from jax.experimental.shard_map import shard_map
    from concourse import bass2jax
    import concourse.mybir as _mybir

    nc = build()
    bass2jax.install_neuronx_cc_hook()

    partition_name = (nc.partition_id_tensor.name
                      if nc.partition_id_tensor else None)
    in_names, out_names, out_avals, zero_shapes = [], [], [], []
    for alloc in nc.m.functions[0].allocations:
        if not isinstance(alloc, _mybir.MemoryLocationSet):
            continue
        name = alloc.memorylocations[0].name
        if alloc.kind == "ExternalInput":
            if name != partition_name:
                in_names.append(name)
        elif alloc.kind == "ExternalOutput":
            out_names.append(name)
            shape = tuple(alloc.tensor_shape)
            dtype = _mybir.dt.np(alloc.dtype)
            out_avals.append(jax.core.ShapedArray(shape, dtype))
            zero_shapes.append((shape, dtype))
    n_params = len(in_names)
    n_outs = len(out_avals)
    all_names = in_names + out_names
    if partition_name is not None:
        all_names = all_names + [partition_name]

    def _body(*args):
        operands = list(args)
        if partition_name is not None:
            operands.append(bass2jax.partition_id_tensor())
        outs = bass2jax._bass_exec_p.bind(
            *operands,
            out_avals=tuple(out_avals),
            in_names=tuple(all_names),
            out_names=tuple(out_names),
            lowering_input_output_aliases=(),
            sim_require_finite=True,
            sim_require_nnan=True,
            nc=nc,
        )
        return tuple(outs)

    devices = jax.devices()[:NCORES]
    mesh = Mesh(np.asarray(devices), ("core",))
    in_specs = (PartitionSpec("core"),) * (n_params + n_outs)
    out_specs = (PartitionSpec("core"),) * n_outs
    donate = tuple(range(n_params, n_params + n_outs))
    sharded = jax.jit(
        shard_map(_body, mesh=mesh, in_specs=in_specs, out_specs=out_specs,
                  check_rep=False),
        donate_argnums=donate, keep_unused=True)

    def _zeros():
        return tuple(jnp.zeros((NCORES * sh[0],) + tuple(sh[1:]), dt)
                     for sh, dt in zero_shapes)

    zeros_fn = jax.jit(
        shard_map(_zeros, mesh=mesh, in_specs=(),
                  out_specs=(PartitionSpec("core"),) * n_outs),
    )
    return nc, sharded, zeros_fn, in_names, out_names, out_avals


def _get_runner():
    global _RUNNER
    if _RUNNER is None:
        _RUNNER = _make_runner()
    return _RUNNER


def _stage_inputs(query, keys, entropy):
    query = np.ascontiguousarray(query, dtype=np.float32)
    keys = np.ascontiguousarray(keys, dtype=np.float32)
    entropy = np.ascontiguousarray(entropy, dtype=np.float32)
    per_core = {
        "q": [query[c * BL:(c + 1) * BL] for c in range(NCORES)],
        "keys": [keys] * NCORES,
        "ent": [entropy[c * NL:(c + 1) * NL] for c in range(NCORES)],
        "entf": [entropy] * NCORES,
    }
    return per_core


def _run_device(per_core, time_exec=False):
    import jax
    nc, sharded, zeros_fn, in_names, out_names, out_avals = _get_runner()
    concat_in = [np.concatenate(per_core[name], axis=0) for name in in_names]
    if time_exec:
        concat_in = [jax.device_put(x) for x in concat_in]
        for x in concat_in:
            x.block_until_ready()
    import time as _time
    zeros = zeros_fn()
    t0 = _time.perf_counter()
    outs = sharded(*concat_in, *zeros)
    jax.block_until_ready(outs)
    t1 = _time.perf_counter()
    res = [{name: np.asarray(outs[i]).reshape(
                (NCORES,) + tuple(out_avals[i].shape))[c]
            for i, name in enumerate(out_names)}
           for c in range(NCORES)]
    return res, (t1 - t0)


def exec_time_s(query, keys, entropy, iters=10):
    """Device-execution wall time (inputs pre-staged on device)."""
    per_core = _stage_inputs(query, keys, entropy)
    _run_device(per_core, time_exec=True)  # warm
    best = float("inf")
    for _ in range(iters):
        _, t = _run_device(per_core, time_exec=True)
        best = min(best, t)
    return best


def kernel(query, keys, entropy):
    per_core = _stage_inputs(query, keys, entropy)
    r, _ = _run_device(per_core)
    uq = np.concatenate([r[c]["uq"] for c in range(NCORES)], axis=0)
    sq = np.concatenate([r[c]["sq"] for c in range(NCORES)], axis=0)
    sm = np.concatenate([r[c]["sm"] for c in range(NCORES)], axis=0)
    um = r[0]["um"]
    dl = np.float32(r[0]["dl"][0, 0])
    sl = np.float32(r[0]["sl"][0, 0])
    return uq, um, sq, sm, dl, sl
